# revision 7
# baseline (speedup 1.0000x reference)
"""GATv2 block (gnn_message_passing) Trainium2 kernel — 8-core SPMD.

Strategy: sort edges by destination, partition destination nodes across the 8
cores (6250 each), process node-groups of 128 per core. Segment softmax +
aggregation become per-group indicator matmuls accumulated in PSUM. Per-edge
operands are fetched with batched SWDGE dma_gather: x[src] via two transposed
gathers from split fp16 tables (int16-index limit, zero-row padding), and
x_r[dst] + relation embedding via one paired gather from an on-device table.
Self-loops (fill_value='mean') use a host-built normalized relation histogram
(integer metadata only) matmul'd with the on-device relation embedding.
"""
import sys

sys.path.insert(0, "/opt/trn_rl_repo")

import numpy as np
import concourse.bacc as bacc
import concourse.bass as bass
import concourse.mybir as mybir
import concourse.tile as tile
from concourse.bass_utils import run_bass_kernel_spmd

F16 = mybir.dt.float16
F32 = mybir.dt.float32
I16 = mybir.dt.int16
AX = mybir.AxisListType
OP = mybir.AluOpType
ACTF = mybir.ActivationFunctionType

CFG_FULL = dict(
    N=50000, E=400000, IN=128, C=32, H=8, ED=64, NREL=512, NCORES=8, SLOPE=0.2,
    SHIFT=3.0,
)
LO_MAX = 32767  # int16 gather-index ceiling; lo table row 32767 is the zero row


def _derive(cfg):
    cfg = dict(cfg)
    cfg["HC"] = cfg["H"] * cfg["C"]
    cfg["NPC"] = cfg["N"] // cfg["NCORES"]
    cfg["NGRP"] = -(-cfg["NPC"] // 128)
    cfg["NPC_PAD"] = cfg["NGRP"] * 128
    cfg["NPAD_TOT"] = cfg["NPC_PAD"] * cfg["NCORES"]
    cfg["NHI"] = cfg["NPAD_TOT"] - LO_MAX + 1  # +1 zero row
    assert cfg["NREL"] % 128 == 0
    cfg["NRELCH"] = cfg["NREL"] // 128
    return cfg


def _wrap16(flat):
    """[G, n] int -> wrapped idx layout [G, 128, n//16]: idx j at [j%16, j//16],
    replicated over the 8 blocks of 16 partitions."""
    G, n = flat.shape
    w = flat.reshape(G, n // 16, 16).transpose(0, 2, 1).astype(np.int16)
    return np.ascontiguousarray(np.tile(w, (1, 8, 1)))


def preprocess(cfg, edge_index, relation_index):
    N, NCORES, NPC, NGRP = cfg["N"], cfg["NCORES"], cfg["NPC"], cfg["NGRP"]
    NREL, NPC_PAD = cfg["NREL"], cfg["NPC_PAD"]
    src = np.asarray(edge_index[0]).astype(np.int64)
    dst = np.asarray(edge_index[1]).astype(np.int64)
    rel = np.asarray(relation_index).astype(np.int64)
    order = np.argsort(dst, kind="stable")
    src_s, dst_s, rel_s = src[order], dst[order], rel[order]
    rowptr = np.searchsorted(dst_s, np.arange(N + 1))

    TPG = 1
    for c in range(NCORES):
        n0 = c * NPC
        for g in range(NGRP):
            ga, gb = n0 + g * 128, min(n0 + (g + 1) * 128, n0 + NPC)
            TPG = max(TPG, -(-(rowptr[gb] - rowptr[ga]) // 128))
    EPG = TPG * 128

    HI_ZERO = cfg["NHI"] - 1
    per_core = []
    for c in range(NCORES):
        n0 = c * NPC
        e0, e1 = rowptr[n0], rowptr[n0 + NPC]
        es, ed, er = src_s[e0:e1], dst_s[e0:e1] - n0, rel_s[e0:e1]
        g_of = ed >> 7
        dloc = ed & 127
        s_of = np.arange(e1 - e0) - (rowptr[n0 + g_of * 128] - e0)
        t_of = s_of >> 7
        p_of = s_of & 127
        jx = t_of * 128 + p_of

        idxlo = np.full((NGRP, EPG), LO_MAX, np.int64)
        idxhi = np.full((NGRP, EPG), HI_ZERO, np.int64)
        lo = es < LO_MAX
        idxlo[g_of[lo], jx[lo]] = es[lo]
        idxhi[g_of[~lo], jx[~lo]] = es[~lo] - LO_MAX

        catx = np.zeros((NGRP, TPG, 2, 128), np.int64)
        catx[:, :, 1, :] = NPC_PAD  # default e row (row 0 of e_aug)
        catx[g_of, t_of, 0, p_of] = g_of * 128 + dloc
        catx[g_of, t_of, 1, p_of] = NPC_PAD + er

        ind = np.zeros((NGRP, 128, EPG), np.float16)
        ind[g_of, p_of, t_of * 128 + dloc] = 1.0

        deg = (rowptr[n0 + 1 : n0 + NPC + 1] - rowptr[n0 : n0 + NPC]).astype(np.float64)
        hn = np.zeros((NPC_PAD, NREL), np.float32)
        np.add.at(hn, (ed, er), 1.0)
        hn[:NPC] /= np.maximum(deg, 1.0)[:, None]
        hnT = (
            hn.reshape(NGRP, 128, cfg["NRELCH"], 128)
            .transpose(0, 3, 2, 1)
            .reshape(NGRP, 128, cfg["NRELCH"] * 128)
            .astype(np.float16)
        )
        per_core.append(
            dict(
                idxlo=_wrap16(idxlo),
                idxhi=_wrap16(idxhi),
                catx=_wrap16(catx.reshape(NGRP, 2 * EPG)),
                ind=ind,
                hnT=np.ascontiguousarray(hnT),
            )
        )
    return per_core, TPG


def build_nc(cfg, TPG, xl_copy_acts=5):
    IN, C, H, HC = cfg["IN"], cfg["C"], cfg["H"], cfg["HC"]
    ED, NREL, NRELCH = cfg["ED"], cfg["NREL"], cfg["NRELCH"]
    NGRP, NPC_PAD = cfg["NGRP"], cfg["NPC_PAD"]
    NHI = cfg["NHI"]
    EPG = TPG * 128
    SHIFT, SLOPE = cfg["SHIFT"], cfg["SLOPE"]

    nc = bacc.Bacc("TRN2", target_bir_lowering=False, debug=False)

    xlo = nc.dram_tensor("xlo", [LO_MAX + 1, IN], F16, kind="ExternalInput")
    xhi = nc.dram_tensor("xhi", [NHI, IN], F16, kind="ExternalInput")
    xloc = nc.dram_tensor("xloc", [NPC_PAD, IN], F16, kind="ExternalInput")
    wl = nc.dram_tensor("wl", [IN, HC], F16, kind="ExternalInput")
    wr = nc.dram_tensor("wr", [IN, HC], F16, kind="ExternalInput")
    relT = nc.dram_tensor("relT", [ED, NREL], F16, kind="ExternalInput")
    we = nc.dram_tensor("we", [ED, HC], F16, kind="ExternalInput")
    bconst = nc.dram_tensor("bconst", [1, HC], F16, kind="ExternalInput")
    bconstb = nc.dram_tensor("bconstb", [128, HC], F16, kind="ExternalInput")
    attb = nc.dram_tensor("attb", [128, HC], F16, kind="ExternalInput")
    biasb = nc.dram_tensor("biasb", [128, C], F32, kind="ExternalInput")
    ident = nc.dram_tensor("ident", [128, 128], F16, kind="ExternalInput")
    negshift = nc.dram_tensor("negshift", [128, 1], F32, kind="ExternalInput")
    ones1 = nc.dram_tensor("ones1", [1, 128], F16, kind="ExternalInput")
    hnT = nc.dram_tensor("hnT", [NGRP, 128, NRELCH * 128], F16, kind="ExternalInput")
    idxlo_d = nc.dram_tensor("idxlo", [NGRP, 128, EPG // 16], I16, kind="ExternalInput")
    idxhi_d = nc.dram_tensor("idxhi", [NGRP, 128, EPG // 16], I16, kind="ExternalInput")
    catx_d = nc.dram_tensor("catx", [NGRP, 128, 2 * EPG // 16], I16, kind="ExternalInput")
    ind_d = nc.dram_tensor("ind", [NGRP, 128, EPG], F16, kind="ExternalInput")

    out_d = nc.dram_tensor("out", [NPC_PAD, C], F32, kind="ExternalOutput")

    cat_d = nc.dram_tensor("cat_t", [NPC_PAD + NREL, HC], F16, kind="Internal")

    with tile.TileContext(nc) as tc:
        with (
            tc.tile_pool(name="const", bufs=1) as cpool,
            tc.tile_pool(name="keep", bufs=NGRP) as kpool,
            tc.tile_pool(name="gwork", bufs=2) as gpool,
            tc.tile_pool(name="ps_xt", bufs=2, space="PSUM") as ps_xt,
            tc.tile_pool(name="ps_xl", bufs=3, space="PSUM") as ps_xl,
            tc.tile_pool(name="ps_se", bufs=2, space="PSUM") as ps_se,
            tc.tile_pool(name="ps_s", bufs=1, space="PSUM") as ps_s,
        ):
            def cload(t, shape, dt):
                s = cpool.tile(shape, dt, tag=t.name)
                nc.sync.dma_start(out=s[:], in_=t[:, :])
                return s

            wl_sb = cload(wl, [IN, HC], F16)
            wr_sb = cload(wr, [IN, HC], F16)
            attb_sb = cload(attb, [128, HC], F16)
            bconst_sb = cload(bconst, [1, HC], F16)
            bconstb_sb = cload(bconstb, [128, HC], F16)
            biasb_sb = cload(biasb, [128, C], F32)
            ident_sb = cload(ident, [128, 128], F16)
            negshift_sb = cload(negshift, [128, 1], F32)
            ones1_sb = cload(ones1, [1, 128], F16)
            relT_sb = cload(relT, [ED, NREL], F16)
            we_sb = cload(we, [ED, HC], F16)

            # relation embedding: e_pure (SBUF, self-loops) + e_aug (-> cat table)
            epure_sb = cpool.tile([128, NRELCH, HC], F16, tag="epure")
            for k in range(NRELCH):
                pse0 = ps_xl.tile([128, HC], F32, tag="ps_xl")
                nc.tensor.matmul(
                    out=pse0[:], lhsT=relT_sb[:, bass.ts(k, 128)], rhs=we_sb[:],
                    start=True, stop=True,
                )
                nc.scalar.copy(out=epure_sb[:, k, :], in_=pse0[:])
                eaug = gpool.tile([128, HC], F16, tag="eaug")
                nc.vector.tensor_add(out=eaug[:], in0=epure_sb[:, k, :], in1=bconstb_sb[:])
                nc.sync.dma_start(
                    out=cat_d[NPC_PAD + k * 128 : NPC_PAD + (k + 1) * 128, :], in_=eaug[:]
                )

            exs_t, msgs_t = [], []

            # ---------- phase 1: node passes ----------
            for g in range(NGRP):
                xg = gpool.tile([128, IN], F16, tag="xg")
                nc.sync.dma_start(out=xg[:], in_=xloc[bass.ts(g, 128), :])
                pxt = ps_xt.tile([128, 128], F16, tag="xt")
                nc.tensor.transpose(out=pxt[:], in_=xg[:], identity=ident_sb[:])
                xt = gpool.tile([128, 128], F16, tag="xt_sb")
                nc.scalar.copy(out=xt[:], in_=pxt[:])

                pxl = ps_xl.tile([128, HC], F32, tag="ps_xl")
                nc.tensor.matmul(out=pxl[:], lhsT=xt[:], rhs=wl_sb[:], start=True, stop=True)
                pxr = ps_xl.tile([128, HC], F32, tag="ps_xl")
                nc.tensor.matmul(out=pxr[:], lhsT=xt[:], rhs=wr_sb[:], start=True, stop=True)
                xl_n = gpool.tile([128, HC], F16, tag="xl_n")
                nc.scalar.copy(out=xl_n[:], in_=pxl[:])
                xr_n = gpool.tile([128, HC], F16, tag="xr_n")
                nc.scalar.copy(out=xr_n[:], in_=pxr[:])
                nc.sync.dma_start(out=cat_d[bass.ts(g, 128), :], in_=xr_n[:])

                hn_sb = gpool.tile([128, NRELCH * 128], F16, tag="hn")
                nc.sync.dma_start(out=hn_sb[:], in_=hnT[g, :, :])
                pse = ps_se.tile([128, HC], F32, tag="ps_se")
                for k in range(NRELCH):
                    nc.tensor.matmul(
                        out=pse[:], lhsT=hn_sb[:, bass.ts(k, 128)],
                        rhs=epure_sb[:, k, :], start=(k == 0), stop=False,
                    )
                nc.tensor.matmul(
                    out=pse[:], lhsT=ones1_sb[:, :], rhs=bconst_sb[:, :],
                    start=False, stop=True,
                )

                ms = gpool.tile([128, HC], F16, tag="ms")
                nc.vector.tensor_add(out=ms[:], in0=xl_n[:], in1=xr_n[:])
                nc.vector.tensor_add(out=ms[:], in0=ms[:], in1=pse[:])
                nc.vector.scalar_tensor_tensor(
                    out=ms[:], in0=ms[:], scalar=SLOPE, in1=ms[:], op0=OP.mult, op1=OP.max
                )
                tmp_s = gpool.tile([128, HC], F16, tag="tmp_s")
                nc.vector.tensor_mul(out=tmp_s[:], in0=ms[:], in1=attb_sb[:])
                logit_s = gpool.tile([128, H], F32, tag="logit_s")
                nc.vector.reduce_sum(
                    out=logit_s[:], in_=tmp_s[:].rearrange("p (h c) -> p h c", c=C),
                    axis=AX.X,
                )
                exs = kpool.tile([128, H], F16, tag="exs")
                nc.scalar.activation(
                    out=exs[:], in_=logit_s[:], func=ACTF.Exp, bias=negshift_sb[:]
                )
                msgs = kpool.tile([128, HC], F16, tag="msgs")
                nc.vector.tensor_mul(
                    out=msgs[:], in0=xl_n[:],
                    in1=exs[:].rearrange("p (h o) -> p h o", o=1).broadcast_to([128, H, C]),
                )
                exs_t.append(exs)
                msgs_t.append(msgs)

            # ---------- phase 2: edge passes ----------
            for g in range(NGRP):
                ilo = gpool.tile([128, EPG // 16], I16, tag="ilo")
                nc.sync.dma_start(out=ilo[:], in_=idxlo_d[g, :, :])
                ihi = gpool.tile([128, EPG // 16], I16, tag="ihi")
                nc.sync.dma_start(out=ihi[:], in_=idxhi_d[g, :, :])
                icat = gpool.tile([128, 2 * EPG // 16], I16, tag="icat")
                nc.sync.dma_start(out=icat[:], in_=catx_d[g, :, :])
                ind_sb = gpool.tile([128, EPG], F16, tag="ind")
                nc.sync.dma_start(out=ind_sb[:], in_=ind_d[g, :, :])

                xtlo = gpool.tile([128, 1, EPG], F16, tag="xtlo")
                nc.gpsimd.dma_gather(
                    out_ap=xtlo[:], in_ap=xlo[:, :], idxs_ap=ilo[:],
                    num_idxs=EPG, num_idxs_reg=EPG, elem_size=IN, transpose=True, single_packet=False,
                )
                xthi = gpool.tile([128, 1, EPG], F16, tag="xthi")
                nc.gpsimd.dma_gather(
                    out_ap=xthi[:], in_ap=xhi[:, :], idxs_ap=ihi[:],
                    num_idxs=EPG, num_idxs_reg=EPG, elem_size=IN, transpose=True, single_packet=False,
                )
                xT = gpool.tile([128, EPG], F16, tag="xT")
                nc.vector.tensor_add(out=xT[:], in0=xtlo[:, 0, :], in1=xthi[:, 0, :])

                xre2 = gpool.tile([128, TPG, 2, HC], F16, tag="xre2")
                nc.gpsimd.dma_gather(
                    out_ap=xre2[:].rearrange("p t two f -> p (t two) f"),
                    in_ap=cat_d[:, :], idxs_ap=icat[:],
                    num_idxs=2 * EPG, num_idxs_reg=2 * EPG, elem_size=HC,
                    transpose=False, single_packet=False,
                )
                xre = gpool.tile([128, TPG, HC], F16, tag="xre")
                nc.vector.tensor_add(out=xre[:], in0=xre2[:, :, 0, :], in1=xre2[:, :, 1, :])

                xl_e = gpool.tile([128, TPG, HC], F16, tag="xl_e")
                for t in range(TPG):
                    pxl_e = ps_xl.tile([128, HC], F32, tag="ps_xl")
                    nc.tensor.matmul(
                        out=pxl_e[:], lhsT=xT[:, bass.ts(t, 128)], rhs=wl_sb[:],
                        start=True, stop=True,
                    )
                    if t < xl_copy_acts:
                        nc.scalar.copy(out=xl_e[:, t, :], in_=pxl_e[:])
                    else:
                        nc.vector.tensor_copy(out=xl_e[:, t, :], in_=pxl_e[:])

                m_g = gpool.tile([128, TPG, HC], F16, tag="m_g")
                nc.vector.tensor_add(out=m_g[:], in0=xl_e[:], in1=xre[:])
                nc.vector.scalar_tensor_tensor(
                    out=m_g[:], in0=m_g[:], scalar=SLOPE, in1=m_g[:], op0=OP.mult,
                    op1=OP.max,
                )
                tmp_g = xre  # reuse
                nc.vector.tensor_mul(
                    out=tmp_g[:], in0=m_g[:],
                    in1=attb_sb[:].rearrange("p (o f) -> p o f", o=1).broadcast_to(
                        [128, TPG, HC]
                    ),
                )
                logits_g = gpool.tile([128, TPG, H], F32, tag="logits_g")
                nc.vector.reduce_sum(
                    out=logits_g[:], in_=tmp_g[:].rearrange("p t (h c) -> p t h c", c=C),
                    axis=AX.X,
                )
                M_g = gpool.tile([128, TPG, 8 + HC], F16, tag="M_g")
                nc.scalar.activation(
                    out=M_g[:, :, 0:8], in_=logits_g[:], func=ACTF.Exp,
                    bias=negshift_sb[:],
                )
                exexp = gpool.tile([128, TPG, HC], F16, tag="exexp")
                nc.scalar.copy(
                    out=exexp[:],
                    in_=M_g[:, :, 0:8]
                    .rearrange("p t (h o) -> p t h o", o=1)
                    .broadcast_to([128, TPG, H, C]),
                )
                nc.vector.tensor_mul(out=M_g[:, :, 8:], in0=xl_e[:], in1=exexp[:])

                ps = ps_s.tile([128, 8 + HC], F32, tag="ps_s")
                for t in range(TPG):
                    nc.tensor.matmul(
                        out=ps[:], lhsT=ind_sb[:, bass.ts(t, 128)], rhs=M_g[:, t, :],
                        start=(t == 0), stop=(t == TPG - 1),
                    )

                den = gpool.tile([128, H], F32, tag="den")
                nc.vector.tensor_add(out=den[:], in0=ps[:, 0:8], in1=exs_t[g][:])
                rec = gpool.tile([128, H], F32, tag="rec")
                nc.vector.reciprocal(out=rec[:], in_=den[:])
                sm = gpool.tile([128, HC], F32, tag="sm")
                nc.vector.tensor_add(out=sm[:], in0=ps[:, 8:], in1=msgs_t[g][:])
                recx = gpool.tile([128, HC], F32, tag="recx")
                nc.scalar.copy(
                    out=recx[:],
                    in_=rec[:].rearrange("p (h o) -> p h o", o=1).broadcast_to([128, H, C]),
                )
                nc.vector.tensor_mul(out=sm[:], in0=sm[:], in1=recx[:])
                f1 = gpool.tile([128, HC // 2], F32, tag="f1")
                nc.vector.tensor_add(out=f1[:], in0=sm[:, 0 : HC // 2], in1=sm[:, HC // 2 :])
                f2 = gpool.tile([128, HC // 4], F32, tag="f2")
                nc.vector.tensor_add(out=f2[:], in0=f1[:, 0 : HC // 4], in1=f1[:, HC // 4 :])
                ot = gpool.tile([128, C], F32, tag="ot")
                nc.vector.tensor_add(out=ot[:], in0=f2[:, 0:C], in1=f2[:, C:])
                nc.vector.scalar_tensor_tensor(
                    out=ot[:], in0=ot[:], scalar=1.0 / H, in1=biasb_sb[:],
                    op0=OP.mult, op1=OP.add,
                )
                nc.sync.dma_start(out=out_d[bass.ts(g, 128), :], in_=ot[:])

    nc.compile()
    return nc


def make_inputs(cfg, TPG, per_core, x, relations, W_l, b_l, W_r, b_r, W_e, att, bias):
    N, IN, C, H, HC = cfg["N"], cfg["IN"], cfg["C"], cfg["H"], cfg["HC"]
    NPC, NPC_PAD, NPAD_TOT, NHI = cfg["NPC"], cfg["NPC_PAD"], cfg["NPAD_TOT"], cfg["NHI"]

    x16 = np.zeros((NPAD_TOT, IN), np.float16)
    x16[:N] = np.asarray(x, np.float32).astype(np.float16)
    xlo = np.zeros((LO_MAX + 1, IN), np.float16)
    xlo[:LO_MAX] = x16[:LO_MAX]
    xhi = np.zeros((NHI, IN), np.float16)
    xhi[: NPAD_TOT - LO_MAX] = x16[LO_MAX:]

    wl16 = np.asarray(W_l, np.float16)
    wr16 = np.asarray(W_r, np.float16)
    relT16 = np.ascontiguousarray(np.asarray(relations, np.float16).T)
    we16 = np.asarray(W_e, np.float16)
    bconst = (np.asarray(b_l, np.float32) + np.asarray(b_r, np.float32)).astype(
        np.float16
    )[None, :]
    bconstb = np.repeat(bconst, 128, 0)
    attb = np.repeat(np.asarray(att, np.float32).reshape(1, HC).astype(np.float16), 128, 0)
    bias_eff = (
        np.asarray(bias, np.float32) + np.asarray(b_l, np.float32).reshape(H, C).mean(0)
    )[None, :]
    biasb = np.repeat(bias_eff, 128, 0).astype(np.float32)
    ident = np.eye(128, dtype=np.float16)
    negshift = np.full((128, 1), -cfg["SHIFT"], np.float32)
    ones1 = np.ones((1, 128), np.float16)

    in_maps = []
    for c in range(cfg["NCORES"]):
        pc = per_core[c]
        n0 = c * NPC
        xloc = np.zeros((NPC_PAD, IN), np.float16)
        xloc[:NPC] = x16[n0 : n0 + NPC]
        in_maps.append(
            dict(
                xlo=xlo, xhi=xhi, xloc=xloc, wl=wl16, wr=wr16, relT=relT16, we=we16,
                bconst=bconst, bconstb=bconstb, attb=attb, biasb=biasb, ident=ident,
                negshift=negshift, ones1=ones1, hnT=pc["hnT"], idxlo=pc["idxlo"],
                idxhi=pc["idxhi"], catx=pc["catx"], ind=pc["ind"],
            )
        )
    return in_maps


_CACHE = {}


def kernel(**inputs):
    cfg = _derive(CFG_FULL)
    per_core, TPG = preprocess(
        cfg, np.asarray(inputs["edge_index"]), np.asarray(inputs["relation_index"])
    )
    if TPG not in _CACHE:
        _CACHE[TPG] = build_nc(cfg, TPG)
    nc = _CACHE[TPG]
    in_maps = make_inputs(
        cfg, TPG, per_core,
        inputs["x"], inputs["relations"], inputs["W_l"], inputs["b_l"],
        inputs["W_r"], inputs["b_r"], inputs["W_e"], inputs["att"], inputs["bias"],
    )
    res = run_bass_kernel_spmd(nc, in_maps, core_ids=list(range(cfg["NCORES"])))
    NPC = cfg["NPC"]
    out = np.concatenate(
        [res.results[c]["out"][:NPC] for c in range(cfg["NCORES"])], 0
    ).astype(np.float32)
    return out, np.asarray(inputs["relations"])


if __name__ == "__main__":
    import reference as ref

    inputs = {k: np.asarray(v) for k, v in ref.setup_inputs().items()}
    expected, _ = ref.reference(**inputs)
    expected = np.asarray(expected)
    actual, _ = kernel(**inputs)
    err = np.abs(actual - expected)
    print("absmax", err.max(), "rel", err.max() / np.abs(expected).max())
